# revision 1
# baseline (speedup 1.0000x reference)
"""Bahdanau-attention LSTM decoder on 8 trn2 NeuronCores.

Sharding: data-parallel over batch B=32 -> 4 per core across 8 cores.
Weights replicated; the decoder-time scan runs locally per shard
(attention softmax reduces over T_enc, which is not sharded).
"""
import numpy as np
import jax
import jax.numpy as jnp

N_CORES = 8
# Problem shapes (hardcoded per contract: kernel.py is self-contained).
B, T_ENC, T_DEC = 32, 1024, 128
ENC_DIM, DEC_DIM, OUT_DIM = 512, 256, 512


def _hard_sigmoid(x):
    return jnp.clip(0.2 * x + 0.5, 0.0, 1.0)


def _decode_shard(enc_output, dec_input, W1, W2, b2, V, W3, b3, Wx, Uh,
                  b_lstm, h0, c0):
    # enc_output: [b, T_enc, enc_dim] local shard; weights replicated.
    xW1 = jnp.einsum("bte,ed->btd", enc_output, W1)
    out_dim = h0.shape[-1]

    def step(carry, x_t):
        h, c = carry
        hW2 = h @ W2 + b2
        u = jnp.tanh(xW1 + hW2[:, None, :])
        scores = jnp.einsum("btd,d->bt", u, V)
        a = jax.nn.softmax(scores, axis=1)
        Xa = jnp.einsum("bt,bte->be", a, enc_output)
        z = jnp.concatenate([x_t, Xa], axis=-1) @ W3 + b3
        gates = z @ Wx + h @ Uh + b_lstm
        i = _hard_sigmoid(gates[:, 0 * out_dim:1 * out_dim])
        f = _hard_sigmoid(gates[:, 1 * out_dim:2 * out_dim])
        g = jnp.tanh(gates[:, 2 * out_dim:3 * out_dim])
        o = _hard_sigmoid(gates[:, 3 * out_dim:4 * out_dim])
        c_new = f * c + i * g
        h_new = o * jnp.tanh(c_new)
        return (h_new, c_new), h_new

    xs = jnp.swapaxes(dec_input, 0, 1)
    _, hs = jax.lax.scan(step, (h0, c0), xs)
    return jnp.swapaxes(hs, 0, 1)


_pmapped = jax.pmap(
    _decode_shard,
    in_axes=(0, 0, None, None, None, None, None, None, None, None, None,
             0, 0),
)

_jitted_single = jax.jit(_decode_shard)


def _run_pmap(inputs):
    per = B // N_CORES
    shard = lambda x: np.ascontiguousarray(
        np.asarray(x).reshape(N_CORES, per, *np.asarray(x).shape[1:]))
    out = _pmapped(
        shard(inputs["enc_output"]), shard(inputs["dec_input"]),
        inputs["W1"], inputs["W2"], inputs["b2"], inputs["V"],
        inputs["W3"], inputs["b3"], inputs["Wx"], inputs["Uh"],
        inputs["b_lstm"], shard(inputs["h0"]), shard(inputs["c0"]))
    out = np.asarray(out)
    return out.reshape(B, T_DEC, OUT_DIM)


def _run_per_device(inputs):
    # Fallback: manual shard across devices with per-device jit calls.
    devs = jax.devices()[:N_CORES]
    per = B // N_CORES
    batch_keys = {"enc_output", "dec_input", "h0", "c0"}
    futs = []
    for i, d in enumerate(devs):
        args = []
        for k in ("enc_output", "dec_input", "W1", "W2", "b2", "V", "W3",
                  "b3", "Wx", "Uh", "b_lstm", "h0", "c0"):
            v = np.asarray(inputs[k])
            if k in batch_keys:
                v = v[i * per:(i + 1) * per]
            args.append(jax.device_put(v, d))
        futs.append(_jitted_single(*args))
    return np.concatenate([np.asarray(f) for f in futs], axis=0)


def kernel(**inputs) -> np.ndarray:
    try:
        out = _run_pmap(inputs)
    except Exception:
        out = _run_per_device(inputs)
    return np.asarray(out, dtype=np.float32)


if __name__ == "__main__":
    rng = np.random.default_rng(0)
    demo = {
        "enc_output": rng.standard_normal((B, T_ENC, ENC_DIM), dtype=np.float32),
        "dec_input": rng.standard_normal((B, T_DEC, DEC_DIM), dtype=np.float32),
        "W1": rng.standard_normal((ENC_DIM, DEC_DIM), dtype=np.float32) * 0.05,
        "W2": rng.standard_normal((OUT_DIM, DEC_DIM), dtype=np.float32) * 0.05,
        "b2": np.zeros((DEC_DIM,), np.float32),
        "V": rng.standard_normal((DEC_DIM,), dtype=np.float32) * 0.05,
        "W3": rng.standard_normal((DEC_DIM + OUT_DIM, OUT_DIM), dtype=np.float32) * 0.05,
        "b3": np.zeros((OUT_DIM,), np.float32),
        "Wx": rng.standard_normal((OUT_DIM, 4 * OUT_DIM), dtype=np.float32) * 0.05,
        "Uh": rng.standard_normal((OUT_DIM, 4 * OUT_DIM), dtype=np.float32) * 0.05,
        "b_lstm": np.zeros((4 * OUT_DIM,), np.float32),
        "h0": np.zeros((B, OUT_DIM), np.float32),
        "c0": np.zeros((B, OUT_DIM), np.float32),
    }
    out = kernel(**demo)
    print("out", out.shape, out.dtype, float(np.abs(out).mean()))

